# revision 34
# baseline (speedup 1.0000x reference)
"""MultiHeadSelfAttentionWithRelativeBias on 8 TRN2 NeuronCores.

Sharding: data-parallel over batch (16 batches -> 2 per core).
v2 pipeline (per batch, fully unrolled Tile program):
  - weights resident in SBUF (bf16); x^T loaded per batch (bf16).
  - V projection packed into per-s-chunk "V_pad" tiles with a ones column
    per head (attention*V also emits softmax row-sums in row 64).
  - Q^T/K^T per pair in bf16 augmented tiles (rows 64:128 = onehot / bias
    features); scores matmul contracts 128 rows so the relative bias is
    free.  All score/AV matmuls in bf16 (f32r streams ~28% slower).
  - attention inner loop is software-pipelined lag-2:
      position p emits: scores(p+1) | exp(p) | next-pair QK-proj mms | av(p-1)
    so each exp has ~8 matmuls of PE time to hide under, and the AV
    accumulator is released a full position before its next use.
  - normalization off the scalar engine: row-sum reciprocal on DVE
    (reciprocal_approx_fast), partition_broadcast on GPSIMD, multiply on
    DVE; un-copy on ACT (which only does exps otherwise).
  - batch b+1's V projection + pair-0 QK proj interleave with batch b's
    O projection so the tail normalize chain hides under matmuls.
"""
import numpy as np
import ml_dtypes

import concourse.bass as bass
import concourse.mybir as mybir
import concourse.tile as tile
from concourse import library_config
from concourse.bass_utils import run_bass_kernel_spmd
from concourse.vector_clock import VectorClock, ScopedClock

# ---------------------------------------------------------------- constants
B, S, E, H, D = 16, 1024, 1024, 16, 64
BOARD = 32
N_CORES = 8
BPC = B // N_CORES  # batches per core
PAIRS = H // 2      # head pairs (128 partition rows per pair)
KC = E // 128       # contraction chunks
F32 = mybir.dt.float32
F32R = mybir.dt.float32r
BF16 = mybir.dt.bfloat16
AF = mybir.ActivationFunctionType

# ------------------------------------------------- walrus compat workarounds


def _patched_drain_and_barrier(self, tick_clock, wait_clock):
    gc = tick_clock.global_clock
    n = len(gc)
    for p in range(n):
        if gc[p] <= 0:
            continue
        sub = VectorClock([0] * n)
        sub.require_at_least(p, gc[p])
        d = self.nc.sync.drain()
        wait_clock.add_sem_waits(d.ins, ScopedClock({None: sub}))
    self.nc.all_engine_barrier()
    popped = self.nc._tile_sem_poison_stack.pop()
    assert popped is self._sem_poison
    self.nc.clear_and_free_semaphores(list(self.sems.allocated().values()))
    self.nc.all_engine_barrier()


tile.TileContext._drain_and_barrier = _patched_drain_and_barrier


def _split_sync_waits(nc, max_waits=1):
    """This container's walrus accepts only one sync-wait per instruction;
    move excess waits onto preceding same-engine NOPs."""
    n_split = 0
    for bb in nc.m.functions[0].blocks:
        insts = bb.instructions
        i = 0
        while i < len(insts):
            inst = insts[i]
            si = inst.sync_info
            if si is not None and si.on_wait and len(si.on_wait) > max_waits:
                waits = list(si.on_wait)
                extra, keep = waits[:-max_waits], waits[-max_waits:]
                nops = []
                for j in range(0, len(extra), max_waits):
                    nops.append(mybir.InstNoOp(
                        name=f"I-{nc.next_id()}",
                        engine=inst.engine,
                        sync_info=mybir.SyncInfo(
                            on_wait=extra[j:j + max_waits], on_update=[]),
                        bass_nofuse=True,
                    ))
                si.on_wait = keep
                inst.sync_info = si
                insts[i:i] = nops
                i += len(nops)
                n_split += 1
            i += 1
    return n_split


# ------------------------------------------------------------- build kernel


def _build_nc():
    nc = bass.Bass("TRN2", target_bir_lowering=False, debug=False,
                   num_devices=1)

    xT = nc.dram_tensor("xT", [BPC, E, S], BF16, kind="ExternalInput")
    wq = nc.dram_tensor("Wq", [E, E], BF16, kind="ExternalInput")
    wk = nc.dram_tensor("Wk", [E, E], BF16, kind="ExternalInput")
    wv = nc.dram_tensor("Wv", [E, E], BF16, kind="ExternalInput")
    wo = nc.dram_tensor("Wo", [E, E], BF16, kind="ExternalInput")
    relb = nc.dram_tensor("relb_sw", [H * 64, S], BF16, kind="ExternalInput")
    onehot = nc.dram_tensor("onehotT", [64, S], BF16, kind="ExternalInput")
    out = nc.dram_tensor("O", [BPC, S, E], F32, kind="ExternalOutput")

    with tile.TileContext(nc) as tc:
        with (
            tc.tile_pool(name="w", bufs=4) as wp,
            tc.tile_pool(name="xt", bufs=2) as xp,
            tc.tile_pool(name="qk", bufs=8) as qkp,
            tc.tile_pool(name="exp", bufs=4) as ep,
            tc.tile_pool(name="vpad", bufs=8) as vp,
            tc.tile_pool(name="outp", bufs=8) as outp_pool,
            tc.tile_pool(name="small", bufs=2) as sp,
            tc.tile_pool(name="osb", bufs=2) as osp,
            tc.tile_pool(name="sc_ps", bufs=2, space="PSUM") as scps,
            tc.tile_pool(name="av_ps", bufs=1, space="PSUM") as avps,
            tc.tile_pool(name="qk_ps", bufs=1, space="PSUM") as qpps,
            tc.tile_pool(name="bc_ps", bufs=1, space="PSUM") as bcps,
        ):
            # resident weights; batch-0 x^T first (it gates the first
            # matmuls).  One batched DMA per tensor: the SP queue issues
            # triggers serially (~0.6us each), so fewer triggers = faster
            # startup.
            # batch-0 x^T and Wv as per-chunk tiles (fine-grained DMA
            # completion: tile-level dependency tracking means the first V
            # matmuls would otherwise wait on whole-tensor DMAs); batch-1
            # x^T and Wk/Wq/Wo batched (one trigger each).
            xt1_buf = xp.tile([128, KC, S], BF16, tag="xt", name="xtb1",
                              bufs=1)
            xt0_tiles = [xp.tile([128, S], BF16, tag="xt0", name=f"xt0_{k}",
                                 bufs=KC)
                         for k in range(KC)]
            xts0 = [t[:] for t in xt0_tiles]
            wt = {}
            wv_tiles = [wp.tile([128, E], BF16, tag="wv", name=f"wv{k}",
                                bufs=KC)
                        for k in range(KC)]
            for k in range(KC):
                wt["v", k] = wv_tiles[k][:]
            wtiles = {}
            for wname in ("k", "q", "o"):
                wtiles[wname] = wp.tile([128, KC, E], BF16, tag="w",
                                        name=f"w{wname}", bufs=3)
                for k in range(KC):
                    wt[wname, k] = wtiles[wname][:, k, :]
            # interleave xt0/wv chunk DMAs on SP (first V matmul ~1.5us in);
            # wk/wq/wo on the idle ACT queue in parallel
            for k in range(KC):
                nc.sync.dma_start(xt0_tiles[k][:],
                                  xT.ap()[0, k * 128:(k + 1) * 128, :])
                nc.sync.dma_start(wv_tiles[k][:],
                                  wv.ap()[k * 128:(k + 1) * 128, :])
            for g in range(KC // 2):
                gsl = slice(2 * g, 2 * g + 2)
                nc.scalar.dma_start(
                    wtiles["k"][:, gsl, :],
                    wk.ap().rearrange("(k p) e -> p k e", p=128)[:, gsl, :])
            nc.scalar.dma_start(
                wtiles["q"][:],
                wq.ap().rearrange("(k p) e -> p k e", p=128))
            nc.scalar.dma_start(
                wtiles["o"][:],
                wo.ap().rearrange("(k p) e -> p k e", p=128))

            ones_sb = sp.tile([1, 64], BF16, tag="ones", name="ones_sb",
                              bufs=1)
            nc.vector.memset(ones_sb[:], 1.0)

            # ---------------- helpers ----------------------------------

            def make_qk_queue(b, m, xts):
                """Allocate qa/ka for pair (b, m) and return (qa, ka, items):
                items = emission callables (bias DMAs, k-proj, casts, q-proj).
                k-proj first so ka is ready for the next pair's first scores."""
                qa = [qkp.tile([128, S], BF16, tag="qk",
                               name=f"qa{b}_{m}_{i}", bufs=8)
                      for i in range(2)]
                ka = [qkp.tile([128, S], BF16, tag="qk",
                               name=f"ka{b}_{m}_{i}", bufs=8)
                      for i in range(2)]
                items = []

                def dmas():
                    for i in range(2):
                        h = 2 * m + i
                        nc.sync.dma_start(qa[i][64:128, :], onehot.ap()[:, :])
                        nc.sync.dma_start(
                            ka[i][64:128, :],
                            relb.ap()[h * 64:(h + 1) * 64, :])
                items.append(dmas)

                for pname, dsts in (("k", ka), ("q", qa)):
                    for n in range(2):
                        nsl = slice(n * 512, (n + 1) * 512)
                        pp = qpps.tile([128, 512], F32, tag="qk_ps",
                                       name=f"{pname}pp{b}_{m}_{n}", bufs=1)
                        for k in range(KC):
                            def mm(pp=pp, nsl=nsl, k=k, pname=pname):
                                nc.tensor.matmul(
                                    pp[:],
                                    wt[pname, k][:, m * 128:(m + 1) * 128],
                                    xts[k][:, nsl], start=(k == 0),
                                    stop=(k == KC - 1))
                            items.append(mm)

                        def casts(pp=pp, dsts=dsts, nsl=nsl):
                            for i in range(2):
                                nc.vector.tensor_copy(
                                    dsts[i][0:64, nsl],
                                    pp[i * 64:(i + 1) * 64, :])
                        items.append(casts)
                return qa, ka, items

            def emit_v_chunk(b, sc, xts, vpads, interleave=None):
                """V projection s-chunk: 16 mms + vpad drain (emitted now)."""
                vtile = vp.tile([128, H * 65], BF16, tag="vpad",
                                name=f"vpad{b}_{sc}", bufs=8)
                pv = scps.tile([128, S], F32, tag="sc_ps",
                               name=f"vps{b}_{sc}", bufs=2)
                for n in range(2):
                    nsl = slice(n * 512, (n + 1) * 512)
                    for k in range(KC):
                        nc.tensor.matmul(
                            pv[:, nsl], xts[k][:, sc * 128:(sc + 1) * 128],
                            wt["v", k][:, nsl], start=(k == 0),
                            stop=(k == KC - 1))
                        if interleave and k % 4 == 3:
                            interleave()
                for n in range(2):
                    off = n * 8 * 65
                    dst = vtile[:, off:off + 8 * 65].rearrange(
                        "p (h d) -> p h d", h=8)[:, :, 0:64]
                    src = pv[:, n * 512:(n + 1) * 512].rearrange(
                        "p (h d) -> p h d", h=8)
                    nc.vector.tensor_copy(dst, src)
                    ones_dst = vtile[:, off:off + 8 * 65].rearrange(
                        "p (h d) -> p h d", h=8)[:, :, 64:65]
                    nc.vector.memset(ones_dst, 1.0)
                vpads.append(vtile)

            def emit_o_mms(b, ms, outps, p_from, p_to, po=None):
                """O projection s-chunk ms, pair contributions [p_from,p_to)."""
                msl = slice(ms * 128, (ms + 1) * 128)
                if po is None:
                    po = scps.tile([128, S], F32, tag="sc_ps",
                                   name=f"ops{b}_{ms}", bufs=2)
                for n in range(2):
                    nsl = slice(n * 512, (n + 1) * 512)
                    for p in range(p_from, p_to):
                        nc.tensor.matmul(
                            po[:, nsl], outps[p][:, msl], wt["o", p][:, nsl],
                            start=(p == 0), stop=(p == PAIRS - 1))
                return po

            def emit_o_drain(b, ms, po):
                msl = slice(ms * 128, (ms + 1) * 128)
                ot = osp.tile([128, S], F32, tag="osb",
                              name=f"ot{b}_{ms}", bufs=2)
                for n in range(2):
                    nsl = slice(n * 512, (n + 1) * 512)
                    nc.scalar.copy(ot[:, nsl], po[:, nsl])
                    nc.sync.dma_start(out.ap()[b, msl, nsl], ot[:, nsl])

            def emit_o_chunk(b, ms, outps):
                po = emit_o_mms(b, ms, outps, 0, PAIRS)
                emit_o_drain(b, ms, po)

            # ---------------- main loop --------------------------------

            xts = xts0
            nbq = []            # (qa, ka) for next batch's pair 0
            queue = []          # pending emission items

            def pop_queue(k=1):
                for _ in range(k):
                    if queue:
                        queue.pop(0)()

            pending_vpads = []

            for b in range(BPC):
                # ---- V phase (batch 0 only; later batches did it in tail)
                if b == 0:
                    vpads = []
                    qa0, ka0, items = make_qk_queue(0, 0, xts)
                    queue.extend(items)
                    for sc in range(KC):
                        emit_v_chunk(0, sc, xts, vpads,
                                     interleave=lambda: pop_queue(1))
                    pop_queue(len(queue))
                    qk_cur = (qa0, ka0)
                else:
                    vpads = pending_vpads
                    qk_cur = nbq.pop()

                # ---- attention: flattened (pair, head) positions --------
                outps = []
                flat = [(m, h2) for m in range(PAIRS) for h2 in range(2)]
                pair_tiles = {0: qk_cur}
                xts_next = []
                sps_cur = None
                av_pending = None
                norm_pending = []
                act_pending = []
                grp = {}

                def emit_scores(qaka, h2, m_, h2_, kc):
                    qa, ka = qaka
                    t = scps.tile([128, S], F32, tag="sc_ps",
                                  name=f"sps{b}_{m_}_{h2_}_{kc}", bufs=2)
                    ksl = slice(kc * 128, (kc + 1) * 128)
                    for n in range(2):
                        nsl = slice(n * 512, (n + 1) * 512)
                        nc.tensor.matmul(t[:, nsl], ka[h2][:, ksl],
                                         qa[h2][:, nsl], start=True, stop=True)
                    return t

                def emit_av(hstate, kc, et):
                    avp, h = hstate["avp"], hstate["h"]
                    for n in range(2):
                        nsl = slice(n * 512, (n + 1) * 512)
                        nc.tensor.matmul(
                            avp[:, nsl],
                            vpads[kc][:, h * 65:(h + 1) * 65],
                            et[:, nsl], start=(kc == 0), stop=(kc == KC - 1))

                GN = 4  # max heads per reciprocal group
                # last two groups small so the final head's normalize chain
                # (which nothing can hide behind on the last batch) is short
                gsched = [4, 4, 4, 3, 1]
                grp["gidx"] = 0
                grp["cnt"] = 0

                def emit_normalize(hstate):
                    """un+sums copy (ACT, frees avp); sums row DMA'd into a
                    group tile.  Per group: one Ln+Exp pair on ACT gives all
                    the group's reciprocals; per-head rec rows DMA'd to
                    partition 0; PE broadcast + DVE multiply deferred."""
                    avp, h = hstate["avp"], hstate["h"]
                    un = sp.tile([65, S], F32, tag="un",
                                 name=f"un{b}_{h}", bufs=GN)
                    nc.scalar.copy(un[:], avp[:, :])
                    hstate["un"] = un
                    gi = grp["cnt"]
                    gs = gsched[grp["gidx"]]
                    if gi == 0:
                        grp["sums"] = sp.tile([GN, S], F32, tag="sums",
                                              name=f"sums{b}_{h}", bufs=2)
                        grp["heads"] = []
                    nc.sync.dma_start(grp["sums"][gi:gi + 1, :],
                                      un[64:65, :])
                    hstate["gi"] = gi
                    grp["heads"].append(hstate)
                    grp["cnt"] = gi + 1
                    if grp["cnt"] == gs:
                        grp["cnt"] = 0
                        grp["gidx"] += 1
                        sums = grp["sums"]
                        heads = grp["heads"]
                        lns = sp.tile([GN, S], F32, tag="lns",
                                      name=f"lns{b}_{h}", bufs=1)
                        rec_g = sp.tile([GN, S], BF16, tag="recg",
                                        name=f"recg{b}_{h}", bufs=1)
                        rec_hs = [sp.tile([1, S], BF16, tag="rech",
                                          name=f"rech{b}_{h}_{i}",
                                          bufs=5)
                                  for i in range(gs)]
                        act_pending.append(
                            lambda: nc.scalar.activation(lns[0:gs, :],
                                                         sums[0:gs, :],
                                                         AF.Ln))
                        act_pending.append(
                            lambda: nc.scalar.activation(rec_g[0:gs, :],
                                                         lns[0:gs, :],
                                                         AF.Exp, scale=-1.0))

                        def rdmas():
                            for i in range(gs):
                                nc.sync.dma_start(rec_hs[i][:],
                                                  rec_g[i:i + 1, :])
                        act_pending.append(rdmas)
                        for hs in heads:
                            def fin(hs=hs, rec_hs=rec_hs, bps_regions=None):
                                rh = rec_hs[hs["gi"]]
                                for n in range(2):
                                    nsl = slice(n * 512, (n + 1) * 512)
                                    if bps_regions:
                                        bps = bps_regions.pop(0)
                                    else:
                                        bps = bcps.tile(
                                            [64, 512], F32, tag="bc_ps",
                                            name=f"bps{b}_{hs['h']}_{n}",
                                            bufs=1)[:]
                                    nc.tensor.matmul(bps, ones_sb[:],
                                                     rh[:, nsl], start=True,
                                                     stop=True)
                                    nc.vector.tensor_mul(
                                        hs["op_t"][hs["h2"] * 64:
                                                   (hs["h2"] + 1) * 64, nsl],
                                        hs["un"][0:64, nsl], bps)
                            norm_pending.append(fin)

                for fi, (m, h2) in enumerate(flat):
                    h = 2 * m + h2
                    if h2 == 0:
                        op_t = outp_pool.tile([128, S], BF16, tag="outp",
                                              name=f"op{b}_{m}", bufs=8)
                        outps.append(op_t)
                        # enqueue the NEXT pair's qk projection work
                        if m + 1 < PAIRS:
                            qa_n, ka_n, items = make_qk_queue(b, m + 1, xts)
                            pair_tiles[m + 1] = (qa_n, ka_n)
                            queue.extend(items)
                        elif b + 1 < BPC:
                            # last pair: prefetch next batch x^T, then its
                            # pair-0 qk proj (lazily, spread via queue)
                            def xdma(b=b):
                                nc.sync.dma_start(
                                    xt1_buf[:],
                                    xT.ap()[b + 1].rearrange(
                                        "(k p) s -> p k s", p=128))
                                xts_next.extend(
                                    xt1_buf[:, k, :] for k in range(KC))
                            queue.append(xdma)

                            def mk():
                                qa2, ka2, its = make_qk_queue(
                                    b + 1, 0, xts_next)
                                nbq.append((qa2, ka2))
                                queue.extend(its)
                            queue.append(mk)

                    hstate = {"m": m, "h2": h2, "h": h, "op_t": outps[m],
                              "avp": avps.tile([65, S], F32, tag="av_ps",
                                               name=f"av{b}_{h}", bufs=1)}
                    if fi == 0:
                        sps_cur = emit_scores(qk_cur, h2, m, h2, 0)

                    for kc in range(KC):
                        # 1. scores for next position
                        if kc + 1 < KC:
                            sps_next = emit_scores(qk_cur, h2, m, h2, kc + 1)
                        elif fi + 1 < len(flat):
                            nm, nh2 = flat[fi + 1]
                            if nh2 == 0:
                                # next pair's qa/ka producers must be emitted
                                # before scores that read them (PE in-order)
                                pop_queue(len(queue))
                            nqk = pair_tiles[nm]
                            sps_next = emit_scores(nqk, nh2, nm, nh2, 0)
                        else:
                            sps_next = None

                        # 2. exp of current position
                        et = ep.tile([128, S], BF16, tag="exp",
                                     name=f"exp{b}_{h}_{kc}", bufs=4)
                        nc.scalar.activation(et[:], sps_cur[:], AF.Exp)

                        # 2b. head-boundary work
                        if kc == 0 and av_pending is not None:
                            pstate, pkc, pet = av_pending
                            emit_av(pstate, pkc, pet)
                            av_pending = None
                            emit_normalize(pstate)
                        elif kc in (1, 2, 3) and act_pending:
                            act_pending.pop(0)()
                        elif kc == 4 and norm_pending:
                            norm_pending.pop(0)()
                        elif kc == 6 and len(norm_pending) > 1:
                            norm_pending.pop(0)()

                        # 3. interleaved qk-proj / prefetch work
                        pop_queue(3 if len(queue) > 12 else 2)

                        # 4. av of previous position (lag 1)
                        if av_pending is not None:
                            emit_av(*av_pending)
                        av_pending = (hstate, kc, et)
                        sps_cur = sps_next

                    if fi + 1 < len(flat) and flat[fi + 1][1] == 0:
                        qk_cur = pair_tiles[flat[fi + 1][0]]

                # drain: final av + normalize of last head
                if av_pending is not None:
                    pstate, pkc, pet = av_pending
                    emit_av(pstate, pkc, pet)
                    av_pending = None
                    emit_normalize(pstate)
                pop_queue(len(queue))
                for fn in act_pending:
                    fn()
                act_pending = []

                # ---- tail: V phase of b+1 interleaved with O phase of b
                pending_vpads = []
                if b + 1 < BPC:
                    for fn in norm_pending:
                        fn()
                    norm_pending = []
                    seq = []
                    vi = oi = 0
                    for i in range(2 * KC):
                        if vi < KC and (oi >= KC or i % 2 == 0 or vi < 2):
                            seq.append(("v", vi)); vi += 1
                        else:
                            seq.append(("o", oi)); oi += 1
                    for kind, idx in seq:
                        if kind == "v":
                            emit_v_chunk(b + 1, idx, xts_next, pending_vpads)
                        else:
                            emit_o_chunk(b, idx, outps)
                    xts = xts_next
                else:
                    # last batch: hide the final normalize chain under the
                    # pair-0..5 contributions of the first two O chunks
                    # (pairs 6-7 are written by the pending normalizes, so
                    # their contributions must be emitted after the fins)
                    po0 = emit_o_mms(b, 0, outps, 0, PAIRS - 2)
                    po1 = emit_o_mms(b, 1, outps, 0, PAIRS - 2)
                    # four spare [64,512] regions in the idle AV accumulator
                    # bank double the broadcast targets so PE bcasts don't
                    # serialize against the DVE muls through the single bcps
                    tail_ps = avps.tile([128, S], F32, tag="av_ps",
                                        name=f"tailps{b}", bufs=1)
                    regions = [tail_ps[0:64, 0:512], tail_ps[0:64, 512:1024],
                               tail_ps[64:128, 0:512],
                               tail_ps[64:128, 512:1024]]
                    for fn in norm_pending:
                        fn(bps_regions=regions)
                    norm_pending = []
                    emit_o_mms(b, 0, outps, PAIRS - 2, PAIRS, po=po0)
                    emit_o_drain(b, 0, po0)
                    emit_o_mms(b, 1, outps, PAIRS - 2, PAIRS, po=po1)
                    emit_o_drain(b, 1, po1)
                    for ms in range(2, KC):
                        emit_o_chunk(b, ms, outps)

    _split_sync_waits(nc)
    return nc


_NC = None


def _get_nc():
    global _NC
    if _NC is None:
        _NC = _build_nc()
    return _NC


# ----------------------------------------------------------- host-side prep


def _host_prep(x, Wq, Wk, Wv, Wo, rel_bias):
    bf = ml_dtypes.bfloat16
    # relative-bias features: for head h, row a (a<32): rel_bias[h, j//32-a+31]
    # row 32+c: rel_bias[h, j%32-c+31]  (j = key index)
    j = np.arange(S)
    jr, jc = j // BOARD, j % BOARD
    a = np.arange(BOARD)
    relb = np.empty((H, 64, S), dtype=np.float32)
    for h in range(H):
        relb[h, 0:32, :] = rel_bias[h][jr[None, :] - a[:, None] + BOARD - 1]
        relb[h, 32:64, :] = rel_bias[h][jc[None, :] - a[:, None] + BOARD - 1]
    relb_sw = relb.reshape(H * 64, S).astype(bf)

    onehot = np.zeros((64, S), dtype=np.float32)
    onehot[jr, j] = 1.0          # rows 0:32 one-hot of q//32
    onehot[32 + jc, j] = 1.0     # rows 32:64 one-hot of q%32
    onehot = onehot.astype(bf)

    wq_b = np.ascontiguousarray((Wq * 0.125).astype(bf))  # fold 1/sqrt(D)
    wk_b = np.ascontiguousarray(Wk.astype(bf))
    wv_b = np.ascontiguousarray(Wv.astype(bf))
    wo_b = np.ascontiguousarray(Wo.astype(bf))

    in_maps = []
    for c in range(N_CORES):
        xc = x[c * BPC:(c + 1) * BPC]                    # [BPC, S, E]
        xt = np.ascontiguousarray(xc.transpose(0, 2, 1).astype(bf))
        in_maps.append({
            "xT": xt, "Wq": wq_b, "Wk": wk_b, "Wv": wv_b, "Wo": wo_b,
            "relb_sw": relb_sw, "onehotT": onehot,
        })
    return in_maps


def kernel(x, Wq, Wk, Wv, Wo, rel_bias, _trace=False):
    nc = _get_nc()
    in_maps = _host_prep(np.asarray(x), np.asarray(Wq), np.asarray(Wk),
                         np.asarray(Wv), np.asarray(Wo), np.asarray(rel_bias))
    res = run_bass_kernel_spmd(nc, in_maps, core_ids=list(range(N_CORES)),
                               trace=_trace)
    out = np.concatenate([res.results[c]["O"] for c in range(N_CORES)], axis=0)
    if _trace:
        kernel.last_exec_time_ns = res.exec_time_ns
        kernel.last_results = res
    return out


# revision 38
# speedup vs baseline: 1.0191x; 1.0191x over previous
"""MultiHeadSelfAttentionWithRelativeBias on 8 TRN2 NeuronCores.

Sharding: data-parallel over batch (16 batches -> 2 per core).
v2 pipeline (per batch, fully unrolled Tile program):
  - weights resident in SBUF (bf16); x^T loaded per batch (bf16).
  - V projection packed into per-s-chunk "V_pad" tiles with a ones column
    per head (attention*V also emits softmax row-sums in row 64).
  - Q^T/K^T per pair in bf16 augmented tiles (rows 64:128 = onehot / bias
    features); scores matmul contracts 128 rows so the relative bias is
    free.  All score/AV matmuls in bf16 (f32r streams ~28% slower).
  - attention inner loop is software-pipelined lag-2:
      position p emits: scores(p+1) | exp(p) | next-pair QK-proj mms | av(p-1)
    so each exp has ~8 matmuls of PE time to hide under, and the AV
    accumulator is released a full position before its next use.
  - normalization off the scalar engine: row-sum reciprocal on DVE
    (reciprocal_approx_fast), partition_broadcast on GPSIMD, multiply on
    DVE; un-copy on ACT (which only does exps otherwise).
  - batch b+1's V projection + pair-0 QK proj interleave with batch b's
    O projection so the tail normalize chain hides under matmuls.
"""
import numpy as np
import ml_dtypes

import concourse.bass as bass
import concourse.mybir as mybir
import concourse.tile as tile
from concourse import library_config
from concourse.bass_utils import run_bass_kernel_spmd
from concourse.vector_clock import VectorClock, ScopedClock

# ---------------------------------------------------------------- constants
B, S, E, H, D = 16, 1024, 1024, 16, 64
BOARD = 32
N_CORES = 8
BPC = B // N_CORES  # batches per core
PAIRS = H // 2      # head pairs (128 partition rows per pair)
KC = E // 128       # contraction chunks
F32 = mybir.dt.float32
F32R = mybir.dt.float32r
BF16 = mybir.dt.bfloat16
AF = mybir.ActivationFunctionType

# ------------------------------------------------- walrus compat workarounds


def _patched_drain_and_barrier(self, tick_clock, wait_clock):
    gc = tick_clock.global_clock
    n = len(gc)
    for p in range(n):
        if gc[p] <= 0:
            continue
        sub = VectorClock([0] * n)
        sub.require_at_least(p, gc[p])
        d = self.nc.sync.drain()
        wait_clock.add_sem_waits(d.ins, ScopedClock({None: sub}))
    self.nc.all_engine_barrier()
    popped = self.nc._tile_sem_poison_stack.pop()
    assert popped is self._sem_poison
    self.nc.clear_and_free_semaphores(list(self.sems.allocated().values()))
    self.nc.all_engine_barrier()


tile.TileContext._drain_and_barrier = _patched_drain_and_barrier


def _split_sync_waits(nc, max_waits=1):
    """This container's walrus accepts only one sync-wait per instruction;
    move excess waits onto preceding same-engine NOPs."""
    n_split = 0
    for bb in nc.m.functions[0].blocks:
        insts = bb.instructions
        i = 0
        while i < len(insts):
            inst = insts[i]
            si = inst.sync_info
            if si is not None and si.on_wait and len(si.on_wait) > max_waits:
                waits = list(si.on_wait)
                extra, keep = waits[:-max_waits], waits[-max_waits:]
                nops = []
                for j in range(0, len(extra), max_waits):
                    nops.append(mybir.InstNoOp(
                        name=f"I-{nc.next_id()}",
                        engine=inst.engine,
                        sync_info=mybir.SyncInfo(
                            on_wait=extra[j:j + max_waits], on_update=[]),
                        bass_nofuse=True,
                    ))
                si.on_wait = keep
                inst.sync_info = si
                insts[i:i] = nops
                i += len(nops)
                n_split += 1
            i += 1
    return n_split


# ------------------------------------------------------------- build kernel


def _build_nc():
    nc = bass.Bass("TRN2", target_bir_lowering=False, debug=False,
                   num_devices=1)

    xT = nc.dram_tensor("xT", [BPC, E, S], BF16, kind="ExternalInput")
    wq = nc.dram_tensor("Wq", [E, E], BF16, kind="ExternalInput")
    wk = nc.dram_tensor("Wk", [E, E], BF16, kind="ExternalInput")
    wv = nc.dram_tensor("Wv", [E, E], BF16, kind="ExternalInput")
    wo = nc.dram_tensor("Wo", [E, E], BF16, kind="ExternalInput")
    relb = nc.dram_tensor("relb_sw", [H * 64, S], BF16, kind="ExternalInput")
    onehot = nc.dram_tensor("onehotT", [64, S], BF16, kind="ExternalInput")
    out = nc.dram_tensor("O", [BPC, S, E], F32, kind="ExternalOutput")

    with tile.TileContext(nc) as tc:
        with (
            tc.tile_pool(name="w", bufs=4) as wp,
            tc.tile_pool(name="xt", bufs=2) as xp,
            tc.tile_pool(name="qk", bufs=8) as qkp,
            tc.tile_pool(name="exp", bufs=4) as ep,
            tc.tile_pool(name="vpad", bufs=8) as vp,
            tc.tile_pool(name="outp", bufs=8) as outp_pool,
            tc.tile_pool(name="small", bufs=2) as sp,
            tc.tile_pool(name="osb", bufs=2) as osp,
            tc.tile_pool(name="sc_ps", bufs=2, space="PSUM") as scps,
            tc.tile_pool(name="av_ps", bufs=1, space="PSUM") as avps,
            tc.tile_pool(name="qk_ps", bufs=1, space="PSUM") as qpps,
            tc.tile_pool(name="bc_ps", bufs=1, space="PSUM") as bcps,
        ):
            # resident weights; batch-0 x^T first (it gates the first
            # matmuls).  One batched DMA per tensor: the SP queue issues
            # triggers serially (~0.6us each), so fewer triggers = faster
            # startup.
            # batch-0 x^T and Wv as per-chunk tiles (fine-grained DMA
            # completion: tile-level dependency tracking means the first V
            # matmuls would otherwise wait on whole-tensor DMAs); batch-1
            # x^T and Wk/Wq/Wo batched (one trigger each).
            xt1_buf = xp.tile([128, KC, S], BF16, tag="xt", name="xtb1",
                              bufs=2)
            xt0_buf = xp.tile([128, KC, S], BF16, tag="xt", name="xtb0",
                              bufs=2)
            xts0 = [xt0_buf[:, k, :] for k in range(KC)]
            wt = {}
            wtiles = {}
            for wname in ("v", "k", "q", "o"):
                wtiles[wname] = wp.tile([128, KC, E], BF16, tag="w",
                                        name=f"w{wname}", bufs=4)
                for k in range(KC):
                    wt[wname, k] = wtiles[wname][:, k, :]
            # interleave xt0/wv 2-chunk piece DMAs on SP;
            # wk/wq/wo on the idle ACT queue in parallel
            for g in range(KC // 2):
                gsl = slice(2 * g, 2 * g + 2)
                nc.sync.dma_start(
                    xt0_buf[:, gsl, :],
                    xT.ap()[0].rearrange("(k p) s -> p k s", p=128)[:, gsl, :])
                nc.sync.dma_start(
                    wtiles["v"][:, gsl, :],
                    wv.ap().rearrange("(k p) e -> p k e", p=128)[:, gsl, :])
            for g in range(KC // 2):
                gsl = slice(2 * g, 2 * g + 2)
                nc.scalar.dma_start(
                    wtiles["k"][:, gsl, :],
                    wk.ap().rearrange("(k p) e -> p k e", p=128)[:, gsl, :])
            nc.scalar.dma_start(
                wtiles["q"][:],
                wq.ap().rearrange("(k p) e -> p k e", p=128))
            nc.scalar.dma_start(
                wtiles["o"][:],
                wo.ap().rearrange("(k p) e -> p k e", p=128))

            ones_sb = sp.tile([1, 64], BF16, tag="ones", name="ones_sb",
                              bufs=1)
            nc.vector.memset(ones_sb[:], 1.0)

            # ---------------- helpers ----------------------------------

            def make_qk_queue(b, m, xts):
                """Allocate qa/ka for pair (b, m) and return (qa, ka, items):
                items = emission callables (bias DMAs, k-proj, casts, q-proj).
                k-proj first so ka is ready for the next pair's first scores."""
                qa = [qkp.tile([128, S], BF16, tag="qk",
                               name=f"qa{b}_{m}_{i}", bufs=8)
                      for i in range(2)]
                ka = [qkp.tile([128, S], BF16, tag="qk",
                               name=f"ka{b}_{m}_{i}", bufs=8)
                      for i in range(2)]
                items = []

                def dmas():
                    for i in range(2):
                        h = 2 * m + i
                        nc.sync.dma_start(qa[i][64:128, :], onehot.ap()[:, :])
                        nc.sync.dma_start(
                            ka[i][64:128, :],
                            relb.ap()[h * 64:(h + 1) * 64, :])
                items.append(dmas)

                for pname, dsts in (("k", ka), ("q", qa)):
                    for n in range(2):
                        nsl = slice(n * 512, (n + 1) * 512)
                        pp = qpps.tile([128, 512], F32, tag="qk_ps",
                                       name=f"{pname}pp{b}_{m}_{n}", bufs=1)
                        for k in range(KC):
                            def mm(pp=pp, nsl=nsl, k=k, pname=pname):
                                nc.tensor.matmul(
                                    pp[:],
                                    wt[pname, k][:, m * 128:(m + 1) * 128],
                                    xts[k][:, nsl], start=(k == 0),
                                    stop=(k == KC - 1))
                            items.append(mm)

                        def casts(pp=pp, dsts=dsts, nsl=nsl):
                            for i in range(2):
                                nc.vector.tensor_copy(
                                    dsts[i][0:64, nsl],
                                    pp[i * 64:(i + 1) * 64, :])
                        items.append(casts)
                return qa, ka, items

            def emit_v_chunk(b, sc, xts, vpads, interleave=None):
                """V projection s-chunk: 16 mms + vpad drain (emitted now)."""
                vtile = vp.tile([128, H * 65], BF16, tag="vpad",
                                name=f"vpad{b}_{sc}", bufs=8)
                pv = scps.tile([128, S], F32, tag="sc_ps",
                               name=f"vps{b}_{sc}", bufs=2)
                for n in range(2):
                    nsl = slice(n * 512, (n + 1) * 512)
                    for k in range(KC):
                        nc.tensor.matmul(
                            pv[:, nsl], xts[k][:, sc * 128:(sc + 1) * 128],
                            wt["v", k][:, nsl], start=(k == 0),
                            stop=(k == KC - 1))
                        if interleave and k % 4 == 3:
                            interleave()
                for n in range(2):
                    off = n * 8 * 65
                    dst = vtile[:, off:off + 8 * 65].rearrange(
                        "p (h d) -> p h d", h=8)[:, :, 0:64]
                    src = pv[:, n * 512:(n + 1) * 512].rearrange(
                        "p (h d) -> p h d", h=8)
                    nc.vector.tensor_copy(dst, src)
                    ones_dst = vtile[:, off:off + 8 * 65].rearrange(
                        "p (h d) -> p h d", h=8)[:, :, 64:65]
                    nc.vector.memset(ones_dst, 1.0)
                vpads.append(vtile)

            def emit_o_mms(b, ms, outps, p_from, p_to, po=None):
                """O projection s-chunk ms, pair contributions [p_from,p_to)."""
                msl = slice(ms * 128, (ms + 1) * 128)
                if po is None:
                    po = scps.tile([128, S], F32, tag="sc_ps",
                                   name=f"ops{b}_{ms}", bufs=2)
                for n in range(2):
                    nsl = slice(n * 512, (n + 1) * 512)
                    for p in range(p_from, p_to):
                        nc.tensor.matmul(
                            po[:, nsl], outps[p][:, msl], wt["o", p][:, nsl],
                            start=(p == 0), stop=(p == PAIRS - 1))
                return po

            def emit_o_drain(b, ms, po):
                msl = slice(ms * 128, (ms + 1) * 128)
                ot = osp.tile([128, S], F32, tag="osb",
                              name=f"ot{b}_{ms}", bufs=2)
                nc.scalar.copy(ot[:], po[:])
                nc.sync.dma_start(out.ap()[b, msl, :], ot[:])

            def emit_o_chunk(b, ms, outps):
                po = emit_o_mms(b, ms, outps, 0, PAIRS)
                emit_o_drain(b, ms, po)

            # ---------------- main loop --------------------------------

            xts = xts0
            nbq = []            # (qa, ka) for next batch's pair 0
            queue = []          # pending emission items

            def pop_queue(k=1):
                for _ in range(k):
                    if queue:
                        queue.pop(0)()

            pending_vpads = []

            for b in range(BPC):
                # ---- V phase (batch 0 only; later batches did it in tail)
                if b == 0:
                    vpads = []
                    qa0, ka0, items = make_qk_queue(0, 0, xts)
                    queue.extend(items)
                    for sc in range(KC):
                        emit_v_chunk(0, sc, xts, vpads,
                                     interleave=lambda: pop_queue(1))
                    pop_queue(len(queue))
                    qk_cur = (qa0, ka0)
                else:
                    vpads = pending_vpads
                    qk_cur = nbq.pop()

                # ---- attention: flattened (pair, head) positions --------
                outps = []
                flat = [(m, h2) for m in range(PAIRS) for h2 in range(2)]
                pair_tiles = {0: qk_cur}
                xts_next = []
                sps_cur = None
                av_pending = None
                norm_pending = []
                act_pending = []
                grp = {}

                def emit_scores(qaka, h2, m_, h2_, kc):
                    qa, ka = qaka
                    t = scps.tile([128, S], F32, tag="sc_ps",
                                  name=f"sps{b}_{m_}_{h2_}_{kc}", bufs=2)
                    ksl = slice(kc * 128, (kc + 1) * 128)
                    for n in range(2):
                        nsl = slice(n * 512, (n + 1) * 512)
                        nc.tensor.matmul(t[:, nsl], ka[h2][:, ksl],
                                         qa[h2][:, nsl], start=True, stop=True)
                    return t

                def emit_av(hstate, kc, et):
                    avp, h = hstate["avp"], hstate["h"]
                    for n in range(2):
                        nsl = slice(n * 512, (n + 1) * 512)
                        nc.tensor.matmul(
                            avp[:, nsl],
                            vpads[kc][:, h * 65:(h + 1) * 65],
                            et[:, nsl], start=(kc == 0), stop=(kc == KC - 1))

                GN = 4  # max heads per reciprocal group
                # last two groups small so the final head's normalize chain
                # (which nothing can hide behind on the last batch) is short
                gsched = [4, 4, 4, 3, 1]
                grp["gidx"] = 0
                grp["cnt"] = 0

                def emit_normalize(hstate):
                    """un+sums copy (ACT, frees avp); sums row DMA'd into a
                    group tile.  Per group: one Ln+Exp pair on ACT gives all
                    the group's reciprocals; per-head rec rows DMA'd to
                    partition 0; PE broadcast + DVE multiply deferred."""
                    avp, h = hstate["avp"], hstate["h"]
                    un = sp.tile([65, S], F32, tag="un",
                                 name=f"un{b}_{h}", bufs=GN)
                    nc.scalar.copy(un[:], avp[:, :])
                    hstate["un"] = un
                    gi = grp["cnt"]
                    gs = gsched[grp["gidx"]]
                    if gi == 0:
                        grp["sums"] = sp.tile([GN, S], F32, tag="sums",
                                              name=f"sums{b}_{h}", bufs=2)
                        grp["heads"] = []
                    nc.sync.dma_start(grp["sums"][gi:gi + 1, :],
                                      un[64:65, :])
                    hstate["gi"] = gi
                    grp["heads"].append(hstate)
                    grp["cnt"] = gi + 1
                    if grp["cnt"] == gs:
                        grp["cnt"] = 0
                        grp["gidx"] += 1
                        sums = grp["sums"]
                        heads = grp["heads"]
                        lns = sp.tile([GN, S], F32, tag="lns",
                                      name=f"lns{b}_{h}", bufs=1)
                        rec_g = sp.tile([GN, S], BF16, tag="recg",
                                        name=f"recg{b}_{h}", bufs=1)
                        rec_hs = [sp.tile([1, S], BF16, tag="rech",
                                          name=f"rech{b}_{h}_{i}",
                                          bufs=5)
                                  for i in range(gs)]
                        act_pending.append(
                            lambda: nc.scalar.activation(lns[0:gs, :],
                                                         sums[0:gs, :],
                                                         AF.Ln))
                        act_pending.append(
                            lambda: nc.scalar.activation(rec_g[0:gs, :],
                                                         lns[0:gs, :],
                                                         AF.Exp, scale=-1.0))

                        def rdmas():
                            for i in range(gs):
                                nc.sync.dma_start(rec_hs[i][:],
                                                  rec_g[i:i + 1, :])
                        act_pending.append(rdmas)
                        for hs in heads:
                            def fin(hs=hs, rec_hs=rec_hs, bps_regions=None):
                                rh = rec_hs[hs["gi"]]
                                for n in range(2):
                                    nsl = slice(n * 512, (n + 1) * 512)
                                    if bps_regions:
                                        bps = bps_regions.pop(0)
                                    else:
                                        bps = bcps.tile(
                                            [64, 512], F32, tag="bc_ps",
                                            name=f"bps{b}_{hs['h']}_{n}",
                                            bufs=1)[:]
                                    nc.tensor.matmul(bps, ones_sb[:],
                                                     rh[:, nsl], start=True,
                                                     stop=True)
                                    nc.vector.tensor_mul(
                                        hs["op_t"][hs["h2"] * 64:
                                                   (hs["h2"] + 1) * 64, nsl],
                                        hs["un"][0:64, nsl], bps)
                            norm_pending.append(fin)

                for fi, (m, h2) in enumerate(flat):
                    h = 2 * m + h2
                    if h2 == 0:
                        op_t = outp_pool.tile([128, S], BF16, tag="outp",
                                              name=f"op{b}_{m}", bufs=8)
                        outps.append(op_t)
                        # enqueue the NEXT pair's qk projection work
                        if m + 1 < PAIRS:
                            qa_n, ka_n, items = make_qk_queue(b, m + 1, xts)
                            pair_tiles[m + 1] = (qa_n, ka_n)
                            queue.extend(items)
                        elif b + 1 < BPC:
                            # last pair: prefetch next batch x^T, then its
                            # pair-0 qk proj (lazily, spread via queue)
                            def xdma(b=b):
                                nc.sync.dma_start(
                                    xt1_buf[:],
                                    xT.ap()[b + 1].rearrange(
                                        "(k p) s -> p k s", p=128))
                                xts_next.extend(
                                    xt1_buf[:, k, :] for k in range(KC))
                            queue.append(xdma)

                            def mk():
                                qa2, ka2, its = make_qk_queue(
                                    b + 1, 0, xts_next)
                                nbq.append((qa2, ka2))
                                queue.extend(its)
                            queue.append(mk)

                    hstate = {"m": m, "h2": h2, "h": h, "op_t": outps[m],
                              "avp": avps.tile([65, S], F32, tag="av_ps",
                                               name=f"av{b}_{h}", bufs=1)}
                    if fi == 0:
                        sps_cur = emit_scores(qk_cur, h2, m, h2, 0)

                    for kc in range(KC):
                        # 1. scores for next position
                        if kc + 1 < KC:
                            sps_next = emit_scores(qk_cur, h2, m, h2, kc + 1)
                        elif fi + 1 < len(flat):
                            nm, nh2 = flat[fi + 1]
                            if nh2 == 0:
                                # next pair's qa/ka producers must be emitted
                                # before scores that read them (PE in-order)
                                pop_queue(len(queue))
                            nqk = pair_tiles[nm]
                            sps_next = emit_scores(nqk, nh2, nm, nh2, 0)
                        else:
                            sps_next = None

                        # 2. exp of current position
                        et = ep.tile([128, S], BF16, tag="exp",
                                     name=f"exp{b}_{h}_{kc}", bufs=4)
                        nc.scalar.activation(et[:], sps_cur[:], AF.Exp)

                        # 2b. head-boundary work
                        if kc == 0 and av_pending is not None:
                            pstate, pkc, pet = av_pending
                            emit_av(pstate, pkc, pet)
                            av_pending = None
                            emit_normalize(pstate)
                        elif kc in (1, 2, 3) and act_pending:
                            act_pending.pop(0)()
                        elif kc == 4 and norm_pending:
                            norm_pending.pop(0)()
                        elif kc == 6 and len(norm_pending) > 1:
                            norm_pending.pop(0)()

                        # 3. interleaved qk-proj / prefetch work
                        pop_queue(3 if len(queue) > 12 else 2)

                        # 4. av of previous position (lag 1)
                        if av_pending is not None:
                            emit_av(*av_pending)
                        av_pending = (hstate, kc, et)
                        sps_cur = sps_next

                    if fi + 1 < len(flat) and flat[fi + 1][1] == 0:
                        qk_cur = pair_tiles[flat[fi + 1][0]]

                # drain: final av + normalize of last head
                if av_pending is not None:
                    pstate, pkc, pet = av_pending
                    emit_av(pstate, pkc, pet)
                    av_pending = None
                    emit_normalize(pstate)
                pop_queue(len(queue))
                for fn in act_pending:
                    fn()
                act_pending = []

                # ---- tail: V phase of b+1 interleaved with O phase of b
                pending_vpads = []
                if b + 1 < BPC:
                    for fn in norm_pending:
                        fn()
                    norm_pending = []
                    seq = []
                    vi = oi = 0
                    for i in range(2 * KC):
                        if vi < KC and (oi >= KC or i % 2 == 0 or vi < 2):
                            seq.append(("v", vi)); vi += 1
                        else:
                            seq.append(("o", oi)); oi += 1
                    for kind, idx in seq:
                        if kind == "v":
                            emit_v_chunk(b + 1, idx, xts_next, pending_vpads)
                        else:
                            emit_o_chunk(b, idx, outps)
                    xts = xts_next
                else:
                    # last batch: hide the final normalize chain under the
                    # pair-0..5 contributions of the first two O chunks
                    # (pairs 6-7 are written by the pending normalizes, so
                    # their contributions must be emitted after the fins)
                    po0 = emit_o_mms(b, 0, outps, 0, PAIRS - 2)
                    po1 = emit_o_mms(b, 1, outps, 0, PAIRS - 2)
                    # four spare [64,512] regions in the idle AV accumulator
                    # bank double the broadcast targets so PE bcasts don't
                    # serialize against the DVE muls through the single bcps
                    tail_ps = avps.tile([128, S], F32, tag="av_ps",
                                        name=f"tailps{b}", bufs=1)
                    regions = [tail_ps[0:64, 0:512], tail_ps[0:64, 512:1024],
                               tail_ps[64:128, 0:512],
                               tail_ps[64:128, 512:1024]]
                    for fn in norm_pending:
                        fn(bps_regions=regions)
                    norm_pending = []
                    emit_o_mms(b, 0, outps, PAIRS - 2, PAIRS, po=po0)
                    emit_o_drain(b, 0, po0)
                    emit_o_mms(b, 1, outps, PAIRS - 2, PAIRS, po=po1)
                    emit_o_drain(b, 1, po1)
                    for ms in range(2, KC):
                        emit_o_chunk(b, ms, outps)

    _split_sync_waits(nc)
    return nc


_NC = None


def _get_nc():
    global _NC
    if _NC is None:
        _NC = _build_nc()
    return _NC


# ----------------------------------------------------------- host-side prep


def _host_prep(x, Wq, Wk, Wv, Wo, rel_bias):
    bf = ml_dtypes.bfloat16
    # relative-bias features: for head h, row a (a<32): rel_bias[h, j//32-a+31]
    # row 32+c: rel_bias[h, j%32-c+31]  (j = key index)
    j = np.arange(S)
    jr, jc = j // BOARD, j % BOARD
    a = np.arange(BOARD)
    relb = np.empty((H, 64, S), dtype=np.float32)
    for h in range(H):
        relb[h, 0:32, :] = rel_bias[h][jr[None, :] - a[:, None] + BOARD - 1]
        relb[h, 32:64, :] = rel_bias[h][jc[None, :] - a[:, None] + BOARD - 1]
    relb_sw = relb.reshape(H * 64, S).astype(bf)

    onehot = np.zeros((64, S), dtype=np.float32)
    onehot[jr, j] = 1.0          # rows 0:32 one-hot of q//32
    onehot[32 + jc, j] = 1.0     # rows 32:64 one-hot of q%32
    onehot = onehot.astype(bf)

    wq_b = np.ascontiguousarray((Wq * 0.125).astype(bf))  # fold 1/sqrt(D)
    wk_b = np.ascontiguousarray(Wk.astype(bf))
    wv_b = np.ascontiguousarray(Wv.astype(bf))
    wo_b = np.ascontiguousarray(Wo.astype(bf))

    in_maps = []
    for c in range(N_CORES):
        xc = x[c * BPC:(c + 1) * BPC]                    # [BPC, S, E]
        xt = np.ascontiguousarray(xc.transpose(0, 2, 1).astype(bf))
        in_maps.append({
            "xT": xt, "Wq": wq_b, "Wk": wk_b, "Wv": wv_b, "Wo": wo_b,
            "relb_sw": relb_sw, "onehotT": onehot,
        })
    return in_maps


def kernel(x, Wq, Wk, Wv, Wo, rel_bias, _trace=False):
    nc = _get_nc()
    in_maps = _host_prep(np.asarray(x), np.asarray(Wq), np.asarray(Wk),
                         np.asarray(Wv), np.asarray(Wo), np.asarray(rel_bias))
    res = run_bass_kernel_spmd(nc, in_maps, core_ids=list(range(N_CORES)),
                               trace=_trace)
    out = np.concatenate([res.results[c]["O"] for c in range(N_CORES)], axis=0)
    if _trace:
        kernel.last_exec_time_ns = res.exec_time_ns
        kernel.last_results = res
    return out
